# revision 9
# baseline (speedup 1.0000x reference)
"""LIF spike (leaky integrate-and-fire) forward kernel for Trainium2.

Recurrence over the time axis T=8 of x[64,128,32,32,8] (fp32):
    u_t = TAU * u_{t-1} * (1 - o_{t-1}) + x_t
    o_t = (u_t > VTH)
Data-parallel over the batch dim: 8 NeuronCores x 8 batches each.

v2 design ("relu/PSUM reset + PE bit-packed output"):
  Exact identity:  TAU * u * (1 - o)  ==  -TAU * P*,  where
      P* = relu(VTH - u) + m''          (PSUM, fp32)
      m'' = (u <= VTH) ? -1.5 : 0       (fp16, exact)
  so each time step needs just ONE op per engine:
      ScalarE:  P*_base = Relu(-u + VTH)      -> PSUM
      GPSIMD:   m'' = (u is_le VTH) * -1.5    -> SBUF fp16
      PE:       P* += I @ m''                 (PSUM accumulate)
      DVE:      u' = (P* * -TAU) + x'         (STT, in-place on x tile)
  The spike outputs are bit-packed on-chip by PE accumulation:
      acc = sum_t 2^t * I @ m_t = M   (m in {0,1})
  and DVE converts -1.5*M -> M into uint8 (one byte per spatial element
  for all 8 time steps), so output DMA is 1 MB/core instead of 8-32 MB.

Two chunks are processed in lockstep in the t-loop so PSUM fits exactly
(2 x P* tiles + 2 x acc tiles = 8 banks at fd=1024) and every engine
stays busy.
"""

import sys

for _p in ("/opt/trn_rl_repo",):
    if _p not in sys.path:
        sys.path.insert(0, _p)

import numpy as np

TAU = 0.1
VTH = 1.5

B, C, H, W, T = 64, 128, 32, 32, 8
NCORES = 8
BS = B // NCORES                      # batches per core
SPAT = BS * C * H * W                 # spatial elems per core per step
P = 128                               # partitions

_compiled = None


def _build_v2(fd: int = 1024, pair: int = 2):
    """pair: how many chunks processed in lockstep (PSUM: pair*(2+2) banks
    of fd=1024 fp32 -- pair=2 fills all 8 banks)."""
    import concourse.bacc as bacc
    import concourse.mybir as mybir
    import concourse.tile as tile

    nch = SPAT // (P * fd)
    f32 = mybir.dt.float32
    f16 = mybir.dt.bfloat16
    u8 = mybir.dt.uint8
    mm = mybir.AluOpType
    relu_f = mybir.ActivationFunctionType.Relu
    copy_f = mybir.ActivationFunctionType.Copy
    sig_f = mybir.ActivationFunctionType.Sigmoid

    nc = bacc.Bacc("TRN2", target_bir_lowering=False, debug=False,
                   num_devices=NCORES)
    x_d = nc.dram_tensor("x", [T * nch * P, fd], f32,
                         kind="ExternalInput").ap()
    o_d = nc.dram_tensor("o", [nch * P, fd], u8, kind="ExternalOutput").ap()

    with tile.TileContext(nc) as tc:
        with (
            tc.tile_pool(name="xp", bufs=4 * pair) as xp,
            tc.tile_pool(name="mp", bufs=3 * pair) as mp,
            tc.tile_pool(name="kp", bufs=2) as kp,
            tc.tile_pool(name="cp", bufs=1) as cp,
            tc.tile_pool(name="pxp", bufs=1, space="PSUM") as pxp,
            tc.tile_pool(name="acp", bufs=1, space="PSUM") as acp,
        ):
            pvth = cp.tile([P, 1], f32, tag="pvth")
            nc.gpsimd.memset(pvth[:], VTH)
            sgb = cp.tile([P, 1], f32, tag="sgb")        # sigmoid bias
            nc.gpsimd.memset(sgb[:], VTH * 1e9)
            wcor = cp.tile([P, P], f16, tag="wcor")      # -1.5 * I
            nc.gpsimd.memset(wcor[:], 0.0)
            nc.gpsimd.affine_select(
                out=wcor[:], in_=wcor[:], compare_op=mm.not_equal,
                fill=-1.5, base=0, pattern=[[-1, P]], channel_multiplier=1)
            wid = cp.tile([P, P], f16, tag="wid")        # 1.0 * I
            nc.gpsimd.memset(wid[:], 0.0)
            nc.gpsimd.affine_select(
                out=wid[:], in_=wid[:], compare_op=mm.not_equal,
                fill=1.0, base=0, pattern=[[-1, P]], channel_multiplier=1)
            wpow = []
            for t in range(T):                            # 2^t * I
                w = cp.tile([P, P], f16, tag=f"wp{t}")
                nc.gpsimd.memset(w[:], 0.0)
                nc.gpsimd.affine_select(
                    out=w[:], in_=w[:], compare_op=mm.not_equal,
                    fill=float(2 ** t), base=0, pattern=[[-1, P]],
                    channel_multiplier=1)
                wpow.append(w)

            nsl = fd // 512                               # psum bank slabs
            # PSUM has_written warmup: matmul is the only engine that sets
            # the per-element has_written bits; a start=False matmul
            # OVERWRITES (not accumulates) elements whose bit is unset.
            # One dummy start=True matmul per P* bank sets the bits once;
            # later engine writes (ScalarE relu) leave them set, so the
            # correction matmuls accumulate correctly. (Verified: without
            # this, the first chunk-pair's t=0 correction drops the relu
            # base entirely.)
            z16 = cp.tile([P, 512], f16, tag="z16")
            nc.gpsimd.memset(z16[:], 0.0)
            for k in range(pair):
                pxw = pxp.tile([P, fd], f32, tag=f"px{k}")
                for j in range(nsl):
                    nc.tensor.matmul(
                        pxw[:, j * 512:(j + 1) * 512], wid[:], z16[:],
                        start=True, stop=True, skip_group_check=True)
            for c0 in range(0, nch, pair):
                cs = range(c0, min(c0 + pair, nch))
                acc = {}
                for c in cs:
                    acc_t = acp.tile([P, fd], f32, tag=f"acc{c % pair}")
                    acc[c] = acc_t
                px = {}
                for t in range(T):
                    ms = {}
                    for c in cs:
                        r0 = (t * nch + c) * P
                        xt = xp.tile([P, fd], f32)
                        nc.sync.dma_start(out=xt[:], in_=x_d[r0:r0 + P, :])
                        if t > 0:
                            # u_t = (P*_{t-1} * -TAU) + x_t  (in place)
                            nc.vector.scalar_tensor_tensor(
                                out=xt[:], in0=px[c][:], scalar=-TAU,
                                in1=xt[:], op0=mm.mult, op1=mm.add)
                        u = xt
                        # m = (u <= VTH) in {0,1} bf16.  Split between
                        # DVE (is_le, 2x mode ~686ns) and ScalarE
                        # (saturating sigmoid, ~1117ns) to balance load.
                        m16 = mp.tile([P, fd], f16)
                        if t % 2 == 0 or (t == 1 and c % 2 == 0):
                            nc.vector.tensor_scalar(
                                m16[:], u[:], VTH, None, mm.is_le)
                        else:
                            nc.scalar.activation(
                                m16[:], u[:], sig_f, bias=sgb[:],
                                scale=-1e9)
                        ms[c] = m16
                        if t < T - 1:
                            # P* base = relu(-u + VTH)  (ScalarE -> PSUM)
                            pxc = pxp.tile([P, fd], f32, tag=f"px{c % pair}")
                            nc.scalar.activation(
                                pxc[:], u[:], relu_f, bias=pvth[:],
                                scale=-1.0)
                            px[c] = pxc
                    # grouped matmuls: all corr (wid) then all pack (wpow[t])
                    if t < T - 1:
                        for c in cs:
                            for j in range(nsl):
                                sl = slice(j * 512, (j + 1) * 512)
                                nc.tensor.matmul(
                                    px[c][:, sl], wcor[:], ms[c][:, sl],
                                    start=False, stop=True,
                                    skip_group_check=True)
                    for c in cs:
                        for j in range(nsl):
                            sl = slice(j * 512, (j + 1) * 512)
                            nc.tensor.matmul(
                                acc[c][:, sl], wpow[t][:], ms[c][:, sl],
                                start=(t == 0), stop=(t == T - 1),
                                skip_group_check=True)
                for c in cs:
                    # acc == M; convert to u8 on ScalarE (reads PSUM)
                    pk8 = kp.tile([P, fd], u8)
                    nc.scalar.activation(
                        pk8[:], acc[c][:], copy_f, bias=0.0, scale=1.0)
                    nc.sync.dma_start(out=o_d[c * P:(c + 1) * P, :],
                                      in_=pk8[:])
    nc.compile()
    nc._lif_fd = fd
    nc._lif_nch = nch
    return nc


def _get_compiled():
    global _compiled
    if _compiled is None:
        _compiled = _build_v2()
    return _compiled


def _shard_tmajor(x: np.ndarray, i: int, fd: int) -> np.ndarray:
    """Core i's shard, time-major rows (t, chunk, p): [T*NCH*P, fd]."""
    xs = x[i * BS:(i + 1) * BS]                     # [BS,C,H,W,T]
    xt = np.moveaxis(xs.reshape(SPAT, T), -1, 0)    # [T, SPAT]
    return np.ascontiguousarray(xt).reshape(T * SPAT // fd, fd)


def kernel(x: np.ndarray, _trace: bool = False):
    nc = _get_compiled()
    from concourse.bass_utils import run_bass_kernel_spmd

    fd = nc._lif_fd
    x = np.asarray(x, dtype=np.float32)
    in_maps = [{"x": _shard_tmajor(x, i, fd)} for i in range(NCORES)]
    res = run_bass_kernel_spmd(
        nc, in_maps, core_ids=list(range(NCORES)), trace=_trace)
    outs = []
    shifts = np.arange(T, dtype=np.uint8)
    for r in res.results:
        m_packed = r["o"].reshape(SPAT, 1)          # M = sum 2^t * m_t
        o = 1.0 - ((m_packed >> shifts) & 1)        # spike = NOT m
        outs.append(o.astype(np.float32).reshape(BS, C, H, W, T))
    out = np.ascontiguousarray(np.concatenate(outs, axis=0))
    if _trace:
        return out, res
    return out


# revision 11
# speedup vs baseline: 1.0991x; 1.0991x over previous
"""LIF spike (leaky integrate-and-fire) forward kernel for Trainium2.

Recurrence over the time axis T=8 of x[64,128,32,32,8] (fp32):
    u_t = TAU * u_{t-1} * (1 - o_{t-1}) + x_t
    o_t = (u_t > VTH)
Data-parallel over the batch dim: 8 NeuronCores x 8 batches each.

v2 design ("relu/PSUM reset + PE bit-packed output"):
  Exact identity:  TAU * u * (1 - o)  ==  -TAU * P*,  where
      P* = relu(VTH - u) + m''          (PSUM, fp32)
      m'' = (u <= VTH) ? -1.5 : 0       (fp16, exact)
  so each time step needs just ONE op per engine:
      ScalarE:  P*_base = Relu(-u + VTH)      -> PSUM
      GPSIMD:   m'' = (u is_le VTH) * -1.5    -> SBUF fp16
      PE:       P* += I @ m''                 (PSUM accumulate)
      DVE:      u' = (P* * -TAU) + x'         (STT, in-place on x tile)
  The spike outputs are bit-packed on-chip by PE accumulation:
      acc = sum_t 2^t * I @ m''_t  = -1.5 * M,  M = sum_t 2^t * m_t
  and DVE converts -1.5*M -> M into uint8 (one byte per spatial element
  for all 8 time steps), so output DMA is 1 MB/core instead of 8-32 MB.

Two chunks are processed in lockstep in the t-loop so PSUM fits exactly
(2 x P* tiles + 2 x acc tiles = 8 banks at fd=1024) and every engine
stays busy.
"""

import sys

for _p in ("/opt/trn_rl_repo",):
    if _p not in sys.path:
        sys.path.insert(0, _p)

import numpy as np

TAU = 0.1
VTH = 1.5

B, C, H, W, T = 64, 128, 32, 32, 8
NCORES = 8
BS = B // NCORES                      # batches per core
SPAT = BS * C * H * W                 # spatial elems per core per step
P = 128                               # partitions

_compiled = None


def _build_v2(fd: int = 1024, pair: int = 2):
    """pair: how many chunks processed in lockstep (PSUM: pair*(2+2) banks
    of fd=1024 fp32 -- pair=2 fills all 8 banks)."""
    import concourse.bacc as bacc
    import concourse.mybir as mybir
    import concourse.tile as tile

    nch = SPAT // (P * fd)
    f32 = mybir.dt.float32
    f16 = mybir.dt.float8e4
    u8 = mybir.dt.uint8
    mm = mybir.AluOpType
    relu_f = mybir.ActivationFunctionType.Relu
    copy_f = mybir.ActivationFunctionType.Copy

    nc = bacc.Bacc("TRN2", target_bir_lowering=False, debug=False,
                   num_devices=NCORES)
    x_d = nc.dram_tensor("x", [T * nch * P, fd], f32,
                         kind="ExternalInput").ap()
    o_d = nc.dram_tensor("o", [nch * P, fd], u8, kind="ExternalOutput").ap()

    with tile.TileContext(nc) as tc:
        with (
            tc.tile_pool(name="xp", bufs=6 * pair) as xp,
            tc.tile_pool(name="mp", bufs=4 * pair) as mp,
            tc.tile_pool(name="kp", bufs=4) as kp,
            tc.tile_pool(name="cp", bufs=1) as cp,
            tc.tile_pool(name="pxp", bufs=1, space="PSUM") as pxp,
            tc.tile_pool(name="acp", bufs=1, space="PSUM") as acp,
        ):
            pvth = cp.tile([P, 1], f32, tag="pvth")
            nc.gpsimd.memset(pvth[:], VTH)
            wid = cp.tile([P, P], f16, tag="wid")        # 1.0 * I
            nc.gpsimd.memset(wid[:], 0.0)
            nc.gpsimd.affine_select(
                out=wid[:], in_=wid[:], compare_op=mm.not_equal,
                fill=1.0, base=0, pattern=[[-1, P]], channel_multiplier=1)
            wpow = []
            for t in range(T):                            # 2^t * I
                w = cp.tile([P, P], f16, tag=f"wp{t}")
                nc.gpsimd.memset(w[:], 0.0)
                nc.gpsimd.affine_select(
                    out=w[:], in_=w[:], compare_op=mm.not_equal,
                    fill=float(2 ** t), base=0, pattern=[[-1, P]],
                    channel_multiplier=1)
                wpow.append(w)

            nsl = fd // 512                               # psum bank slabs
            # PSUM has_written warmup: matmul is the only engine that sets
            # the per-element has_written bits; a start=False matmul
            # OVERWRITES (not accumulates) elements whose bit is unset.
            # One dummy start=True matmul per P* bank sets the bits once;
            # later engine writes (ScalarE relu) leave them set, so the
            # correction matmuls accumulate correctly. (Verified: without
            # this, the first chunk-pair's t=0 correction drops the relu
            # base entirely.)
            z16 = cp.tile([P, 512], f16, tag="z16")
            nc.gpsimd.memset(z16[:], 0.0)
            for k in range(pair):
                pxw = pxp.tile([P, fd], f32, tag=f"px{k}")
                for j in range(nsl):
                    nc.tensor.matmul(
                        pxw[:, j * 512:(j + 1) * 512], wid[:], z16[:],
                        start=True, stop=True, skip_group_check=True)
            for c0 in range(0, nch, pair):
                cs = range(c0, min(c0 + pair, nch))
                acc = {}
                for c in cs:
                    acc_t = acp.tile([P, fd], f32, tag=f"acc{c % pair}")
                    acc[c] = acc_t
                px = {}
                for t in range(T):
                    ms = {}
                    for c in cs:
                        r0 = (t * nch + c) * P
                        xt = xp.tile([P, fd], f32)
                        nc.sync.dma_start(out=xt[:], in_=x_d[r0:r0 + P, :])
                        if t > 0:
                            # u_t = (P*_{t-1} * -TAU) + x_t  (in place)
                            nc.vector.scalar_tensor_tensor(
                                out=xt[:], in0=px[c][:], scalar=-TAU,
                                in1=xt[:], op0=mm.mult, op1=mm.add)
                        u = xt
                        # m'' = (u <= VTH) * -1.5   (bf16, exact; DVE 2x)
                        m16 = mp.tile([P, fd], f16)
                        nc.vector.tensor_scalar(
                            m16[:], u[:], VTH, -1.5, mm.is_le, mm.mult)
                        ms[c] = m16
                        if t < T - 1:
                            # P* base = relu(-u + VTH)  (ScalarE -> PSUM)
                            pxc = pxp.tile([P, fd], f32, tag=f"px{c % pair}")
                            nc.scalar.activation(
                                pxc[:], u[:], relu_f, bias=pvth[:],
                                scale=-1.0)
                            px[c] = pxc
                    # grouped matmuls: all corr (wid) then all pack (wpow[t])
                    if t < T - 1:
                        for c in cs:
                            for j in range(nsl):
                                sl = slice(j * 512, (j + 1) * 512)
                                nc.tensor.matmul(
                                    px[c][:, sl], wid[:], ms[c][:, sl],
                                    start=False, stop=True,
                                    skip_group_check=True)
                    for c in cs:
                        for j in range(nsl):
                            sl = slice(j * 512, (j + 1) * 512)
                            nc.tensor.matmul(
                                acc[c][:, sl], wpow[t][:], ms[c][:, sl],
                                start=(t == 0), stop=(t == T - 1),
                                skip_group_check=True)
                for c in cs:
                    # acc = -1.5*M  ->  M (u8) on ScalarE (reads PSUM)
                    pk8 = kp.tile([P, fd], u8)
                    nc.scalar.activation(
                        pk8[:], acc[c][:], copy_f, bias=0.0,
                        scale=-1.0 / 1.5)
                    nc.sync.dma_start(out=o_d[c * P:(c + 1) * P, :],
                                      in_=pk8[:])
    nc.compile()
    nc._lif_fd = fd
    nc._lif_nch = nch
    return nc


def _get_compiled():
    global _compiled
    if _compiled is None:
        _compiled = _build_v2()
    return _compiled


def _shard_tmajor(x: np.ndarray, i: int, fd: int) -> np.ndarray:
    """Core i's shard, time-major rows (t, chunk, p): [T*NCH*P, fd]."""
    xs = x[i * BS:(i + 1) * BS]                     # [BS,C,H,W,T]
    xt = np.moveaxis(xs.reshape(SPAT, T), -1, 0)    # [T, SPAT]
    return np.ascontiguousarray(xt).reshape(T * SPAT // fd, fd)


def kernel(x: np.ndarray, _trace: bool = False):
    nc = _get_compiled()
    from concourse.bass_utils import run_bass_kernel_spmd

    fd = nc._lif_fd
    x = np.asarray(x, dtype=np.float32)
    in_maps = [{"x": _shard_tmajor(x, i, fd)} for i in range(NCORES)]
    res = run_bass_kernel_spmd(
        nc, in_maps, core_ids=list(range(NCORES)), trace=_trace)
    outs = []
    shifts = np.arange(T, dtype=np.uint8)
    for r in res.results:
        m_packed = r["o"].reshape(SPAT, 1)          # M = sum 2^t * m_t
        o = 1.0 - ((m_packed >> shifts) & 1)        # spike = NOT m
        outs.append(o.astype(np.float32).reshape(BS, C, H, W, T))
    out = np.ascontiguousarray(np.concatenate(outs, axis=0))
    if _trace:
        return out, res
    return out
